# revision 34
# baseline (speedup 1.0000x reference)
"""Trainium2 Bass kernel for nn_ModelConTT_46016279609475 (TT interpolation).

y[b] = v0[b]^T V1[b] V2[b] v3[b], where v_i are linearly-interpolated slices
of tiny TT cores at per-point grid coordinates derived from x[b, :].

Strategy (per NeuronCore, data-parallel over B):
  * Precompute joint tables on device with PE matmuls (bf16 operands):
      G[n0, n1, k] = sum_c core0[n0, c] * core1[c, n1, k]      (u-side)
      H[n3, n2, k] = sum_c core3[c, n3] * core2[k, n2, c]      (v-side)
    stored fp16 in DRAM as 4-corner-packed 256B entries; the entry layout is
      G4[(n0*128+n1), (dn0, dn1, k)] fp16 in the first 128B, 128B pad,
    so one dma_gather element (256B minimum) fetches everything needed for
    the bilinear interpolation of u[b] (and same for v[b]).  fp16 halves the
    table-write DMA vs f32 and enables 2x-mode DVE in the combine.
  * Per point: idx = lo0*128 + lo1 (int16), one 256B dma_gather per table,
    then the 4-corner weighted sum and the final k-dot:
      y[b] = sum_k (sum_c wG_c gG[c,k]) * (sum_c wH_c gH[c,k])
    The corner weights are expanded over k into fp16 on the Activation
    engine (keeps every DVE op in 2x mode); the H-side corner multiply runs
    on GpSimd to keep the DVE stage under the gather-pair rate.

Batch mapping per core: shard b of size 32768; on-chip layout is
"p-minor": element i lives at partition i%128, free col i//128, matching
dma_gather's output layout dst[i%128, i//128]. Index lists are mod-16
wrapped as dma_gather requires (idx for i at [i%16, i//16]) and replicated
across all 8 Q7 core groups (each SWDGE core pair reads its own 16 rows).

Exact-floor trick (f32-safe): t = (xc + 2^23) - 2^23 rounds to nearest;
g = (t > xc); floor = t - g; frac = xc - floor computed via the exact
(t1 - 2^23) path to avoid re-rounding.
"""

import numpy as np
import ml_dtypes

import concourse.bass as bass
import concourse.bacc as bacc
import concourse.mybir as mybir
import concourse.tile as tile
from concourse import library_config
from concourse.bass_utils import run_bass_kernel_spmd

F32 = mybir.dt.float32
F16 = mybir.dt.float16
BF16 = mybir.dt.bfloat16
I16 = mybir.dt.int16
OP = mybir.AluOpType
AF = mybir.ActivationFunctionType

NCORES = 8
B = 262144
BS = B // NCORES          # 32768 points per core
P = 128                   # partitions
J = BS // P               # 256 free cols per partition
NCH = 8                   # pipeline chunks
JC = J // NCH             # 32 cols per chunk
NIDX = P * JC             # 4096 idxs per gather
LC = NIDX // 16           # 256 idx-list cols per chunk
N = 128                   # mode size
R = 16                    # TT rank
TE = N * N                # table entries
ES = 128                  # fp16 elems per entry: 4 corners x 16 k + 64 pad = 256B
ED = 64                   # fp16 elems of real data per entry
MAGIC = float(2 ** 23)
SCALE = (N - 1) / 2.0     # 63.5
M16 = BS // 16            # 2048 idx-list cols total

_CACHED = None
DEBUG_TILES = {}


def _build_nc(stage="full"):
    nc = bacc.Bacc("TRN2")

    x_pm = nc.dram_tensor("x_pm", [P, J, 4], F32, kind="ExternalInput")
    xq = nc.dram_tensor("xq", [64, M16 // 2, 2], F32, kind="ExternalInput")
    c0t = nc.dram_tensor("c0t", [16, 129], BF16, kind="ExternalInput")
    c1f = nc.dram_tensor("c1f", [16, 2096], BF16, kind="ExternalInput")
    c3f = nc.dram_tensor("c3f", [16, 129], BF16, kind="ExternalInput")
    c2t = nc.dram_tensor("c2t", [16, 2096], BF16, kind="ExternalInput")
    y_pm = nc.dram_tensor("y_pm", [P, J], F32, kind="ExternalOutput")

    with tile.TileContext(nc) as tc:
        with (
            tc.tile_pool(name="per", bufs=1) as pe,
            tc.tile_pool(name="ps", bufs=1, space="PSUM") as pp,
            tc.tile_pool(name="dr", bufs=1, space="DRAM") as dp,
        ):
            nc.gpsimd.load_library(library_config.mlp)

            # persistent tiles (lists fully memset once: the gather idx AP
            # spans all 128 partitions but HW only reads rows 0-31, its
            # queue's core pair; the sim reads rows 0-15)
            LG = pe.tile([P, M16], I16)
            LH = pe.tile([P, M16], I16)
            # no memsets: the gather idx APs span 128 partitions but HW only
            # reads rows 0-31 (its queue's core pair) and the sim rows 0-15,
            # all of which the band STTs + replication copies fully write.
            WG = pe.tile([P, 4, J], F32)
            WH = pe.tile([P, 4, J], F32)
            M4 = pe.tile([P, N, ES], F16)
            ysb = pe.tile([P, J], F32)
            # merged table: entry cell = [G-data 128B | H-data 128B]; the
            # H gather reads 256B starting at the H half (its tail spills
            # into the next entry's G half and is never read), so one extra
            # pad entry keeps the last H read in bounds.
            m4d = dp.tile([TE + 1, ES], F16)

            with tc.tile_pool(name="pre", bufs=1) as wp:
                # tiny warmup activation so LoadActFuncSet (1.3us) runs at
                # t=0 instead of right before the first table-pack copy.
                warm = wp.tile([1, 1], F32)
                nc.scalar.activation(warm[:], ysb[0:1, 0:1], AF.Copy,
                                     bias=0.0, scale=0.0)

                # ------------- constant loads -------------
                # core matrices first: they gate the PE matmuls -> Act packs
                # -> table writes, the longest head chain; the index lists
                # (from xq) have ~9us of slack behind it.
                c0t_s = wp.tile([16, 129], BF16)
                nc.sync.dma_start(c0t_s[:], c0t[:])
                c1f_s = wp.tile([16, 2096], BF16)
                nc.sync.dma_start(c1f_s[:], c1f[:])
                c3f_s = wp.tile([16, 129], BF16)
                nc.sync.dma_start(c3f_s[:], c3f[:])
                c2t_s = wp.tile([16, 2096], BF16)
                nc.sync.dma_start(c2t_s[:], c2t[:])
                xq_s = wp.tile([112, M16], F32)
                xqv = xq[:].rearrange("p a b -> p (a b)")
                nc.sync.dma_start(xq_s[0:16, :], xqv[0:16, :])
                nc.sync.dma_start(xq_s[32:48, :], xqv[16:32, :])
                nc.sync.dma_start(xq_s[64:80, :], xqv[32:48, :])
                nc.sync.dma_start(xq_s[96:112, :], xqv[48:64, :])

                # ------------- index lists (DVE + Act) ----
                # four 16-row bands (at partition bases 0/32/64/96 -- the
                # only legal compute starts): G cols 0-1023 / G cols
                # 1024-2047 / H cols 0-1023 / H cols 1024-2047. Halves the
                # per-op free size vs a single band.
                nc.vector.tensor_scalar(
                    xq_s[:], xq_s[:], SCALE, SCALE, OP.mult, OP.add
                )
                # t1q on GpSimd: Act is saturated by table packs and DVE by
                # the rest of this chain; Pool is idle until the gathers.
                t1q = wp.tile([112, M16], F32)
                nc.gpsimd.tensor_scalar(
                    t1q[:], xq_s[:], 1.0, MAGIC, OP.mult, OP.add
                )
                gq = wp.tile([112, M16], F32)
                nc.vector.scalar_tensor_tensor(
                    gq[:], t1q[:], -MAGIC, xq_s[:], OP.add, OP.is_gt
                )
                # lo = (t1 - MAGIC) - g  (exact floor), in place over t1q
                nc.vector.scalar_tensor_tensor(
                    t1q[:], t1q[:], -MAGIC, gq[:], OP.add, OP.subtract
                )
                # idx = lo_hi*128 + lo_lo, int16 cast fused into the op's
                # output dtype; written straight into the list tiles.
                lo_hi = t1q[:].rearrange("p (m two) -> p m two", two=2)
                H16 = M16 // 2
                for band, dst in (
                    (0, LG[0:16, 0:H16]),
                    (32, LG[32:48, H16:M16]),
                    (64, LH[64:80, 0:H16]),
                    (96, LH[96:112, H16:M16]),
                ):
                    nc.vector.scalar_tensor_tensor(
                        dst,
                        lo_hi[band : band + 16, :, 0],
                        128.0,
                        lo_hi[band : band + 16, :, 1],
                        OP.mult,
                        OP.add,
                    )
                # (the LG/LH replication copies are emitted after the table
                # build: they wait on the band STTs, and putting them ahead
                # of the table writes in the SP DMA queue head-of-line
                # blocks the writes for ~3us each)

                # ------------- table build ----------------
                # chunk-outer so each n1-quarter's DRAM write starts as soon
                # as its corner-pack copies land (overlaps write with build).
                # Entry cell (256B) = [G 4-corner c-major [c, k] fp16 128B |
                # H likewise 128B]: both tables in one dense DRAM write.
                # G packs on Activation, H packs on GpSimd (parallel);
                # DVE stays free for the index lists / interp weights.
                tgv = M4[:, :, 0:ED].rearrange("p n (c k) -> p n c k", k=R)
                thv = M4[:, :, ED:ES].rearrange("p n (c k) -> p n c k", k=R)
                tdrv = m4d[0:TE, :].rearrange("(p a) b -> p a b", p=P)
                for ch in range(4):
                    for tag, dst8, lhs, rhs in (
                        ("mmG", tgv, c0t_s, c1f_s),
                        ("mmH", thv, c3f_s, c2t_s),
                    ):
                        psq = pp.tile([P, 4, 512], F32, tag=tag)
                        for ci, (dhi, dlo) in enumerate(
                            ((0, 0), (0, 1), (1, 0), (1, 1))
                        ):
                            nc.tensor.matmul(
                                psq[:, ci],
                                lhs[:, dhi : dhi + 128],
                                rhs[
                                    :,
                                    16 * dlo + 512 * ch : 16 * dlo + 512 * ch + 512,
                                ],
                                start=True,
                                stop=True,
                            )
                        # pack: psq [p, c, (n1 32, k 16)] -> tbl [p, n1, c, k]
                        # (GPSIMD cannot read PSUM.) Activation's serial
                        # pack chain binds the last quarter write, so the
                        # final quarter goes to DVE, which is free once the
                        # index lists are done.
                        src = psq[:].rearrange("p c (a k) -> p a c k", k=R)
                        if ch == 3 and tag == "mmH":
                            # the 8th pack goes to DVE (free once the index
                            # lists are done) so Act's serial chain is 7 long
                            nc.vector.tensor_copy(
                                dst8[:, 32 * ch : 32 * ch + 32], src
                            )
                        else:
                            nc.scalar.copy(dst8[:, 32 * ch : 32 * ch + 32], src)
                    nc.sync.dma_start(
                        tdrv[:, 32 * ch : 32 * ch + 32, :],
                        M4[:, 32 * ch : 32 * ch + 32, :],
                    )

                # index-list replication on the gpsimd SWDGE queue: the SP
                # HWDGE queue carries the table writes, and these copies
                # (which gate the first gather's descriptor generation)
                # must not queue behind them.
                nc.gpsimd.dma_start(LG[0:16, H16:M16], LG[32:48, H16:M16])
                nc.gpsimd.dma_start(LG[16:32, :], LG[0:16, :])
                nc.gpsimd.dma_start(LH[0:16, 0:H16], LH[64:80, 0:H16])
                nc.gpsimd.dma_start(LH[0:16, H16:M16], LH[96:112, H16:M16])
                nc.gpsimd.dma_start(LH[16:32, :], LH[0:16, :])

                # ------------- gather + combine ---------------
                # Software-pipelined emission: the first gather pair is
                # emitted BEFORE the interp-weight chain so the Tile
                # framework's coarse semaphore batching doesn't make the
                # gathers wait on the (irrelevant) weight computation.
                if stage != "full":
                    nc.vector.memset(ysb[:], 0.0)
                # full-width chunks except the last, split 16/8/8 so the
                # post-final-gather combine tail is short.
                segs = [(JC * c, JC) for c in range(NCH - 1)]
                j4 = JC * (NCH - 1)
                segs += [(j4, 16), (j4 + 16, 8), (j4 + 24, 8)]
                if stage == "tables":
                    segs = []
                elif stage == "gather1":
                    segs = segs[:1]
                gsrc = m4d[0:TE, :]
                hsrc = (
                    m4d[:]
                    .rearrange("a b -> (a b)")[ED : ED + TE * ES]
                    .rearrange("(a b) -> a b", b=ES)
                )
                with (
                    tc.tile_pool(name="gbuf", bufs=3) as gb,
                    tc.tile_pool(name="cbuf", bufs=2) as cb,
                ):
                    gathered = []

                    def emit_gather(si):
                        j0, w = segs[si]
                        nidx = P * w
                        lc0, lc1 = 8 * j0, 8 * (j0 + w)
                        gGt = gb.tile([P, JC, ES], F16, tag="gG")
                        nc.gpsimd.dma_gather(
                            gGt[:, 0:w], gsrc, LG[:, lc0:lc1], nidx, nidx, ES,
                            queue_num=0, single_packet=False,
                        )
                        gHt = gb.tile([P, JC, ES], F16, tag="gH")
                        nc.gpsimd.dma_gather(
                            gHt[:, 0:w], hsrc, LH[:, lc0:lc1], nidx, nidx, ES,
                            queue_num=0, single_packet=False,
                        )
                        gathered.append((gGt, gHt))

                    if segs:
                        emit_gather(0)

                    # ------------- interp weights -------------
                    # x_s is [128, (256 j, 4 d)]; w = frac(xc), a = 1 - w.
                    # (loaded here, after the build, so its DMA doesn't
                    # delay the first table-quarter write)
                    x_s = wp.tile([P, J * 4], F32)
                    nc.sync.dma_start(x_s[:], x_pm[:].rearrange("p a b -> p (a b)"))
                    nc.vector.tensor_scalar(
                        x_s[:], x_s[:], SCALE, SCALE, OP.mult, OP.add
                    )
                    t1 = wp.tile([P, J * 4], F32)
                    nc.scalar.activation(
                        t1[:], x_s[:], AF.Copy, bias=MAGIC, scale=1.0
                    )
                    gw = wp.tile([P, J * 4], F32)
                    nc.vector.scalar_tensor_tensor(
                        gw[:], t1[:], -MAGIC, x_s[:], OP.add, OP.is_gt
                    )
                    # s1 = (t1 - MAGIC) - xc = t - xc  (t1 - MAGIC is exact)
                    s1 = wp.tile([P, J * 4], F32)
                    nc.vector.scalar_tensor_tensor(
                        s1[:], t1[:], -MAGIC, x_s[:], OP.add, OP.subtract
                    )
                    # w = g - (t - xc) = xc - floor(xc), in place over s1
                    nc.vector.tensor_tensor(s1[:], gw[:], s1[:], OP.subtract)
                    aw = wp.tile([P, J * 4], F32, tag="t1")
                    nc.vector.tensor_scalar(
                        aw[:], s1[:], -1.0, 1.0, OP.mult, OP.add
                    )

                    wv = s1[:].rearrange("p (j d) -> p j d", d=4)
                    av = aw[:].rearrange("p (j d) -> p j d", d=4)
                    # G corners (dn0, dn1): (a0,a1),(a0,w1),(w0,a1),(w0,w1)
                    nc.vector.tensor_tensor(WG[:, 0, :], av[:, :, 0], av[:, :, 1], OP.mult)
                    nc.vector.tensor_tensor(WG[:, 1, :], av[:, :, 0], wv[:, :, 1], OP.mult)
                    nc.vector.tensor_tensor(WG[:, 2, :], wv[:, :, 0], av[:, :, 1], OP.mult)
                    nc.vector.tensor_tensor(WG[:, 3, :], wv[:, :, 0], wv[:, :, 1], OP.mult)
                    # H corners (dn3, dn2): (a3,a2),(a3,w2),(w3,a2),(w3,w2)
                    nc.vector.tensor_tensor(WH[:, 0, :], av[:, :, 3], av[:, :, 2], OP.mult)
                    nc.vector.tensor_tensor(WH[:, 1, :], av[:, :, 3], wv[:, :, 2], OP.mult)
                    nc.vector.tensor_tensor(WH[:, 2, :], wv[:, :, 3], av[:, :, 2], OP.mult)
                    nc.vector.tensor_tensor(WH[:, 3, :], wv[:, :, 3], wv[:, :, 2], OP.mult)

                    for si, (j0, w) in enumerate(segs):
                        if si + 1 < len(segs):
                            emit_gather(si + 1)
                        gGt, gHt = gathered[si]

                        # fp16 weights expanded over k on Activation (keeps
                        # the DVE multiplies in 2x mode: packed stride-1).
                        uv = []
                        for ti, (g, W) in enumerate(((gGt, WG), (gHt, WH))):
                            wxt = cb.tile([P, JC, 4, R], F16, tag=f"wx{ti}")
                            wx = wxt[:, 0:w]
                            nc.scalar.copy(
                                wx,
                                W[:, :, j0 : j0 + w]
                                .rearrange("p c j -> p j c")
                                .unsqueeze(3)
                                .broadcast_to([P, w, 4, R]),
                            )
                            # m[j, c, k] = corner value * corner weight
                            gv = g[:, 0:w, 0:ED].rearrange(
                                "p j (c k) -> p j c k", c=4
                            )
                            mt = cb.tile([P, JC, 4, R], F16, tag=f"m{ti}")
                            m = mt[:, 0:w]
                            nc.vector.tensor_tensor(m, gv, wx, OP.mult)
                            t2t = cb.tile([P, JC, 2, R], F16, tag=f"t{ti}")
                            t2 = t2t[:, 0:w]
                            nc.vector.tensor_tensor(
                                t2, m[:, :, 0:2], m[:, :, 2:4], OP.add
                            )
                            ut = cb.tile([P, JC, R], F16, tag=f"u{ti}")
                            u = ut[:, 0:w]
                            nc.vector.tensor_tensor(
                                u, t2[:, :, 0], t2[:, :, 1], OP.add
                            )
                            uv.append(u)

                        prt = cb.tile([P, JC, R], F16, tag="pr")
                        pr = prt[:, 0:w]
                        nc.vector.tensor_tensor(pr, uv[0], uv[1], OP.mult)
                        nc.vector.tensor_reduce(
                            ysb[:, j0 : j0 + w],
                            pr,
                            mybir.AxisListType.X,
                            OP.add,
                        )

            nc.sync.dma_start(y_pm[:], ysb[:])
            DEBUG_TILES.update(LG=LG, LH=LH, WG=WG, WH=WH, M4=M4,
                               ysb=ysb, m4d=m4d)

    nc.finalize()
    return nc


def _prep_inputs(x, core0, core1, core2, core3):
    """Host-side input marshalling: shard x over cores, lay out tensors in
    the on-chip layouts the kernel expects, pad core matrices for the
    shifted-corner matmuls (cast to bf16 on host)."""
    xs = np.ascontiguousarray(np.asarray(x, dtype=np.float32).reshape(NCORES, BS, 4))

    core0 = np.asarray(core0, dtype=np.float32)
    core1 = np.asarray(core1, dtype=np.float32)
    core2 = np.asarray(core2, dtype=np.float32)
    core3 = np.asarray(core3, dtype=np.float32)

    c0 = core0[0]                        # [128, 16]
    c0t = np.ascontiguousarray(
        np.concatenate([c0.T, c0.T[:, -1:]], axis=1)
    ).astype(ml_dtypes.bfloat16)
    c1 = core1.reshape(16, 2048)
    c1f = np.ascontiguousarray(
        np.concatenate([c1, np.tile(c1[:, -16:], (1, 3))], axis=1)
    ).astype(ml_dtypes.bfloat16)
    c2 = np.ascontiguousarray(core2.transpose(2, 1, 0)).reshape(16, 2048)
    c2t = np.ascontiguousarray(
        np.concatenate([c2, np.tile(c2[:, -16:], (1, 3))], axis=1)
    ).astype(ml_dtypes.bfloat16)
    c3 = core3[:, :, 0]                  # [16, 128]
    c3f = np.ascontiguousarray(
        np.concatenate([c3, c3[:, -1:]], axis=1)
    ).astype(ml_dtypes.bfloat16)

    in_maps = []
    for c in range(NCORES):
        xc_ = xs[c]
        x_pm = np.ascontiguousarray(
            xc_.reshape(J, P, 4).transpose(1, 0, 2)
        )  # [128, 256, 4]
        xg = np.ascontiguousarray(
            xc_[:, [0, 1]].reshape(M16, 16, 2).transpose(1, 0, 2)
        )  # [16, 2048, 2]
        xh = np.ascontiguousarray(
            xc_[:, [3, 2]].reshape(M16, 16, 2).transpose(1, 0, 2)
        )
        H16 = M16 // 2
        xq = np.concatenate(
            [xg[:, :H16], xg[:, H16:], xh[:, :H16], xh[:, H16:]], axis=0
        )  # [64, 1024, 2]
        in_maps.append(
            {
                "x_pm": x_pm,
                "xq": xq,
                "c0t": c0t,
                "c1f": c1f,
                "c3f": c3f,
                "c2t": c2t,
            }
        )
    return in_maps


def kernel(x, core0, core1, core2, core3):
    global _CACHED
    if _CACHED is None:
        _CACHED = _build_nc()
    nc = _CACHED
    in_maps = _prep_inputs(x, core0, core1, core2, core3)
    res = run_bass_kernel_spmd(nc, in_maps, core_ids=list(range(NCORES)))
    outs = []
    for c in range(NCORES):
        y_pm = res.results[c]["y_pm"]          # [128, 256]
        outs.append(np.ascontiguousarray(np.asarray(y_pm).T).reshape(-1))
    return np.concatenate(outs).astype(np.float32)


# revision 35
# speedup vs baseline: 1.0130x; 1.0130x over previous
"""Trainium2 Bass kernel for nn_ModelConTT_46016279609475 (TT interpolation).

y[b] = v0[b]^T V1[b] V2[b] v3[b], where v_i are linearly-interpolated slices
of tiny TT cores at per-point grid coordinates derived from x[b, :].

Strategy (per NeuronCore, data-parallel over B):
  * Precompute joint tables on device with PE matmuls (bf16 operands):
      G[n0, n1, k] = sum_c core0[n0, c] * core1[c, n1, k]      (u-side)
      H[n3, n2, k] = sum_c core3[c, n3] * core2[k, n2, c]      (v-side)
    stored fp16 in DRAM as 4-corner-packed 256B entries; the entry layout is
      G4[(n0*128+n1), (dn0, dn1, k)] fp16 in the first 128B, 128B pad,
    so one dma_gather element (256B minimum) fetches everything needed for
    the bilinear interpolation of u[b] (and same for v[b]).  fp16 halves the
    table-write DMA vs f32 and enables 2x-mode DVE in the combine.
  * Per point: idx = lo0*128 + lo1 (int16), one 256B dma_gather per table,
    then the 4-corner weighted sum and the final k-dot:
      y[b] = sum_k (sum_c wG_c gG[c,k]) * (sum_c wH_c gH[c,k])
    The corner weights are expanded over k into fp16 on the Activation
    engine (keeps every DVE op in 2x mode); the H-side corner multiply runs
    on GpSimd to keep the DVE stage under the gather-pair rate.

Batch mapping per core: shard b of size 32768; on-chip layout is
"p-minor": element i lives at partition i%128, free col i//128, matching
dma_gather's output layout dst[i%128, i//128]. Index lists are mod-16
wrapped as dma_gather requires (idx for i at [i%16, i//16]) and replicated
across all 8 Q7 core groups (each SWDGE core pair reads its own 16 rows).

Exact-floor trick (f32-safe): t = (xc + 2^23) - 2^23 rounds to nearest;
g = (t > xc); floor = t - g; frac = xc - floor computed via the exact
(t1 - 2^23) path to avoid re-rounding.
"""

import numpy as np
import ml_dtypes

import concourse.bass as bass
import concourse.bacc as bacc
import concourse.mybir as mybir
import concourse.tile as tile
from concourse import library_config
from concourse.bass_utils import run_bass_kernel_spmd

F32 = mybir.dt.float32
F16 = mybir.dt.float16
BF16 = mybir.dt.bfloat16
I16 = mybir.dt.int16
OP = mybir.AluOpType
AF = mybir.ActivationFunctionType

NCORES = 8
B = 262144
BS = B // NCORES          # 32768 points per core
P = 128                   # partitions
J = BS // P               # 256 free cols per partition
NCH = 8                   # pipeline chunks
JC = J // NCH             # 32 cols per chunk
NIDX = P * JC             # 4096 idxs per gather
LC = NIDX // 16           # 256 idx-list cols per chunk
N = 128                   # mode size
R = 16                    # TT rank
TE = N * N                # table entries
ES = 128                  # fp16 elems per entry: 4 corners x 16 k + 64 pad = 256B
ED = 64                   # fp16 elems of real data per entry
MAGIC = float(2 ** 23)
SCALE = (N - 1) / 2.0     # 63.5
M16 = BS // 16            # 2048 idx-list cols total

_CACHED = None
DEBUG_TILES = {}


def _build_nc(stage="full"):
    nc = bacc.Bacc("TRN2")

    x_pm = nc.dram_tensor("x_pm", [P, J, 4], F32, kind="ExternalInput")
    xq = nc.dram_tensor("xq", [64, M16 // 2, 2], F32, kind="ExternalInput")
    c0t = nc.dram_tensor("c0t", [16, 129], BF16, kind="ExternalInput")
    c1f = nc.dram_tensor("c1f", [16, 2096], BF16, kind="ExternalInput")
    c3f = nc.dram_tensor("c3f", [16, 129], BF16, kind="ExternalInput")
    c2t = nc.dram_tensor("c2t", [16, 2096], BF16, kind="ExternalInput")
    y_pm = nc.dram_tensor("y_pm", [P, J], F32, kind="ExternalOutput")

    with tile.TileContext(nc) as tc:
        with (
            tc.tile_pool(name="per", bufs=1) as pe,
            tc.tile_pool(name="ps", bufs=1, space="PSUM") as pp,
            tc.tile_pool(name="dr", bufs=1, space="DRAM") as dp,
        ):
            nc.gpsimd.load_library(library_config.mlp)

            # persistent tiles (lists fully memset once: the gather idx AP
            # spans all 128 partitions but HW only reads rows 0-31, its
            # queue's core pair; the sim reads rows 0-15)
            LG = pe.tile([P, M16], I16)
            LH = pe.tile([P, M16], I16)
            # no memsets: the gather idx APs span 128 partitions but HW only
            # reads rows 0-31 (its queue's core pair) and the sim rows 0-15,
            # all of which the band STTs + replication copies fully write.
            WG = pe.tile([P, 4, J], F32)
            WH = pe.tile([P, 4, J], F32)
            M4 = pe.tile([P, N, ES], F16)
            ysb = pe.tile([P, J], F32)
            # merged table: entry cell = [G-data 128B | H-data 128B]; the
            # H gather reads 256B starting at the H half (its tail spills
            # into the next entry's G half and is never read), so one extra
            # pad entry keeps the last H read in bounds.
            m4d = dp.tile([TE + 1, ES], F16)

            with tc.tile_pool(name="pre", bufs=1) as wp:
                # tiny warmup activation so LoadActFuncSet (1.3us) runs at
                # t=0 instead of right before the first table-pack copy.
                warm = wp.tile([1, 1], F32)
                nc.scalar.activation(warm[:], ysb[0:1, 0:1], AF.Copy,
                                     bias=0.0, scale=0.0)

                # ------------- constant loads -------------
                # core matrices first: they gate the PE matmuls -> Act packs
                # -> table writes, the longest head chain; the index lists
                # (from xq) have ~9us of slack behind it.
                c0t_s = wp.tile([16, 129], BF16)
                nc.sync.dma_start(c0t_s[:], c0t[:])
                c1f_s = wp.tile([16, 2096], BF16)
                nc.sync.dma_start(c1f_s[:], c1f[:])
                c3f_s = wp.tile([16, 129], BF16)
                nc.sync.dma_start(c3f_s[:], c3f[:])
                c2t_s = wp.tile([16, 2096], BF16)
                nc.sync.dma_start(c2t_s[:], c2t[:])
                xq_s = wp.tile([112, M16], F32)
                xqv = xq[:].rearrange("p a b -> p (a b)")
                nc.sync.dma_start(xq_s[0:16, :], xqv[0:16, :])
                nc.sync.dma_start(xq_s[32:48, :], xqv[16:32, :])
                nc.sync.dma_start(xq_s[64:80, :], xqv[32:48, :])
                nc.sync.dma_start(xq_s[96:112, :], xqv[48:64, :])

                # ------------- index lists (DVE + Act) ----
                # four 16-row bands (at partition bases 0/32/64/96 -- the
                # only legal compute starts): G cols 0-1023 / G cols
                # 1024-2047 / H cols 0-1023 / H cols 1024-2047. Halves the
                # per-op free size vs a single band.
                nc.vector.tensor_scalar(
                    xq_s[:], xq_s[:], SCALE, SCALE, OP.mult, OP.add
                )
                # t1q on DVE: Act is saturated by table packs.
                t1q = wp.tile([112, M16], F32)
                nc.vector.tensor_scalar(
                    t1q[:], xq_s[:], 1.0, MAGIC, OP.mult, OP.add
                )
                gq = wp.tile([112, M16], F32)
                nc.vector.scalar_tensor_tensor(
                    gq[:], t1q[:], -MAGIC, xq_s[:], OP.add, OP.is_gt
                )
                # lo = (t1 - MAGIC) - g  (exact floor), in place over t1q
                nc.vector.scalar_tensor_tensor(
                    t1q[:], t1q[:], -MAGIC, gq[:], OP.add, OP.subtract
                )
                # idx = lo_hi*128 + lo_lo, int16 cast fused into the op's
                # output dtype; written straight into the list tiles.
                lo_hi = t1q[:].rearrange("p (m two) -> p m two", two=2)
                H16 = M16 // 2
                for band, dst in (
                    (0, LG[0:16, 0:H16]),
                    (32, LG[32:48, H16:M16]),
                    (64, LH[64:80, 0:H16]),
                    (96, LH[96:112, H16:M16]),
                ):
                    nc.vector.scalar_tensor_tensor(
                        dst,
                        lo_hi[band : band + 16, :, 0],
                        128.0,
                        lo_hi[band : band + 16, :, 1],
                        OP.mult,
                        OP.add,
                    )
                # (the LG/LH replication copies are emitted after the table
                # build: they wait on the band STTs, and putting them ahead
                # of the table writes in the SP DMA queue head-of-line
                # blocks the writes for ~3us each)

                # ------------- table build ----------------
                # chunk-outer so each n1-quarter's DRAM write starts as soon
                # as its corner-pack copies land (overlaps write with build).
                # Entry cell (256B) = [G 4-corner c-major [c, k] fp16 128B |
                # H likewise 128B]: both tables in one dense DRAM write.
                # G packs on Activation, H packs on GpSimd (parallel);
                # DVE stays free for the index lists / interp weights.
                tgv = M4[:, :, 0:ED].rearrange("p n (c k) -> p n c k", k=R)
                thv = M4[:, :, ED:ES].rearrange("p n (c k) -> p n c k", k=R)
                tdrv = m4d[0:TE, :].rearrange("(p a) b -> p a b", p=P)
                for ch in range(4):
                    for tag, dst8, lhs, rhs in (
                        ("mmG", tgv, c0t_s, c1f_s),
                        ("mmH", thv, c3f_s, c2t_s),
                    ):
                        psq = pp.tile([P, 4, 512], F32, tag=tag)
                        for ci, (dhi, dlo) in enumerate(
                            ((0, 0), (0, 1), (1, 0), (1, 1))
                        ):
                            nc.tensor.matmul(
                                psq[:, ci],
                                lhs[:, dhi : dhi + 128],
                                rhs[
                                    :,
                                    16 * dlo + 512 * ch : 16 * dlo + 512 * ch + 512,
                                ],
                                start=True,
                                stop=True,
                            )
                        # pack: psq [p, c, (n1 32, k 16)] -> tbl [p, n1, c, k]
                        # (GPSIMD cannot read PSUM.) Activation's serial
                        # pack chain binds the last quarter write, so the
                        # final quarter goes to DVE, which is free once the
                        # index lists are done.
                        src = psq[:].rearrange("p c (a k) -> p a c k", k=R)
                        if ch == 3 and tag == "mmH":
                            # the 8th pack goes to DVE (free once the index
                            # lists are done) so Act's serial chain is 7 long
                            nc.vector.tensor_copy(
                                dst8[:, 32 * ch : 32 * ch + 32], src
                            )
                        else:
                            nc.scalar.copy(dst8[:, 32 * ch : 32 * ch + 32], src)
                    nc.sync.dma_start(
                        tdrv[:, 32 * ch : 32 * ch + 32, :],
                        M4[:, 32 * ch : 32 * ch + 32, :],
                    )

                # index-list replication on the gpsimd SWDGE queue: the SP
                # HWDGE queue carries the table writes, and these copies
                # (which gate the first gather's descriptor generation)
                # must not queue behind them.
                nc.gpsimd.dma_start(LG[0:16, H16:M16], LG[32:48, H16:M16])
                nc.gpsimd.dma_start(LG[16:32, :], LG[0:16, :])
                nc.gpsimd.dma_start(LH[0:16, 0:H16], LH[64:80, 0:H16])
                nc.gpsimd.dma_start(LH[0:16, H16:M16], LH[96:112, H16:M16])
                nc.gpsimd.dma_start(LH[16:32, :], LH[0:16, :])

                # ------------- gather + combine ---------------
                # Software-pipelined emission: the first gather pair is
                # emitted BEFORE the interp-weight chain so the Tile
                # framework's coarse semaphore batching doesn't make the
                # gathers wait on the (irrelevant) weight computation.
                if stage != "full":
                    nc.vector.memset(ysb[:], 0.0)
                # full-width chunks except the last, split 16/8/8 so the
                # post-final-gather combine tail is short.
                segs = [(JC * c, JC) for c in range(NCH - 1)]
                j4 = JC * (NCH - 1)
                segs += [(j4, 16), (j4 + 16, 8), (j4 + 24, 8)]
                if stage == "tables":
                    segs = []
                elif stage == "gather1":
                    segs = segs[:1]
                gsrc = m4d[0:TE, :]
                hsrc = (
                    m4d[:]
                    .rearrange("a b -> (a b)")[ED : ED + TE * ES]
                    .rearrange("(a b) -> a b", b=ES)
                )
                with (
                    tc.tile_pool(name="gbuf", bufs=3) as gb,
                    tc.tile_pool(name="cbuf", bufs=2) as cb,
                ):
                    gathered = []

                    def emit_gather(si):
                        j0, w = segs[si]
                        nidx = P * w
                        lc0, lc1 = 8 * j0, 8 * (j0 + w)
                        gGt = gb.tile([P, JC, ES], F16, tag="gG")
                        nc.gpsimd.dma_gather(
                            gGt[:, 0:w], gsrc, LG[:, lc0:lc1], nidx, nidx, ES,
                            queue_num=0, single_packet=False,
                        )
                        gHt = gb.tile([P, JC, ES], F16, tag="gH")
                        nc.gpsimd.dma_gather(
                            gHt[:, 0:w], hsrc, LH[:, lc0:lc1], nidx, nidx, ES,
                            queue_num=0, single_packet=False,
                        )
                        gathered.append((gGt, gHt))

                    if segs:
                        emit_gather(0)

                    # ------------- interp weights -------------
                    # x_s is [128, (256 j, 4 d)]; w = frac(xc), a = 1 - w.
                    # (loaded here, after the build, so its DMA doesn't
                    # delay the first table-quarter write)
                    x_s = wp.tile([P, J * 4], F32)
                    nc.sync.dma_start(x_s[:], x_pm[:].rearrange("p a b -> p (a b)"))
                    nc.vector.tensor_scalar(
                        x_s[:], x_s[:], SCALE, SCALE, OP.mult, OP.add
                    )
                    t1 = wp.tile([P, J * 4], F32)
                    nc.scalar.activation(
                        t1[:], x_s[:], AF.Copy, bias=MAGIC, scale=1.0
                    )
                    gw = wp.tile([P, J * 4], F32)
                    nc.vector.scalar_tensor_tensor(
                        gw[:], t1[:], -MAGIC, x_s[:], OP.add, OP.is_gt
                    )
                    # s1 = (t1 - MAGIC) - xc = t - xc  (t1 - MAGIC is exact)
                    s1 = wp.tile([P, J * 4], F32)
                    nc.vector.scalar_tensor_tensor(
                        s1[:], t1[:], -MAGIC, x_s[:], OP.add, OP.subtract
                    )
                    # w = g - (t - xc) = xc - floor(xc), in place over s1
                    nc.vector.tensor_tensor(s1[:], gw[:], s1[:], OP.subtract)
                    aw = wp.tile([P, J * 4], F32, tag="t1")
                    nc.vector.tensor_scalar(
                        aw[:], s1[:], -1.0, 1.0, OP.mult, OP.add
                    )

                    wv = s1[:].rearrange("p (j d) -> p j d", d=4)
                    av = aw[:].rearrange("p (j d) -> p j d", d=4)
                    # G corners (dn0, dn1): (a0,a1),(a0,w1),(w0,a1),(w0,w1)
                    nc.vector.tensor_tensor(WG[:, 0, :], av[:, :, 0], av[:, :, 1], OP.mult)
                    nc.vector.tensor_tensor(WG[:, 1, :], av[:, :, 0], wv[:, :, 1], OP.mult)
                    nc.vector.tensor_tensor(WG[:, 2, :], wv[:, :, 0], av[:, :, 1], OP.mult)
                    nc.vector.tensor_tensor(WG[:, 3, :], wv[:, :, 0], wv[:, :, 1], OP.mult)
                    # H corners (dn3, dn2): (a3,a2),(a3,w2),(w3,a2),(w3,w2)
                    nc.vector.tensor_tensor(WH[:, 0, :], av[:, :, 3], av[:, :, 2], OP.mult)
                    nc.vector.tensor_tensor(WH[:, 1, :], av[:, :, 3], wv[:, :, 2], OP.mult)
                    nc.vector.tensor_tensor(WH[:, 2, :], wv[:, :, 3], av[:, :, 2], OP.mult)
                    nc.vector.tensor_tensor(WH[:, 3, :], wv[:, :, 3], wv[:, :, 2], OP.mult)

                    for si, (j0, w) in enumerate(segs):
                        if si + 1 < len(segs):
                            emit_gather(si + 1)
                        gGt, gHt = gathered[si]

                        # fp16 weights expanded over k on Activation (keeps
                        # the DVE multiplies in 2x mode: packed stride-1).
                        uv = []
                        for ti, (g, W) in enumerate(((gGt, WG), (gHt, WH))):
                            wxt = cb.tile([P, JC, 4, R], F16, tag=f"wx{ti}")
                            wx = wxt[:, 0:w]
                            nc.scalar.copy(
                                wx,
                                W[:, :, j0 : j0 + w]
                                .rearrange("p c j -> p j c")
                                .unsqueeze(3)
                                .broadcast_to([P, w, 4, R]),
                            )
                            # m[j, c, k] = corner value * corner weight
                            gv = g[:, 0:w, 0:ED].rearrange(
                                "p j (c k) -> p j c k", c=4
                            )
                            mt = cb.tile([P, JC, 4, R], F16, tag=f"m{ti}")
                            m = mt[:, 0:w]
                            nc.vector.tensor_tensor(m, gv, wx, OP.mult)
                            t2t = cb.tile([P, JC, 2, R], F16, tag=f"t{ti}")
                            t2 = t2t[:, 0:w]
                            nc.vector.tensor_tensor(
                                t2, m[:, :, 0:2], m[:, :, 2:4], OP.add
                            )
                            ut = cb.tile([P, JC, R], F16, tag=f"u{ti}")
                            u = ut[:, 0:w]
                            nc.vector.tensor_tensor(
                                u, t2[:, :, 0], t2[:, :, 1], OP.add
                            )
                            uv.append(u)

                        prt = cb.tile([P, JC, R], F16, tag="pr")
                        pr = prt[:, 0:w]
                        nc.vector.tensor_tensor(pr, uv[0], uv[1], OP.mult)
                        nc.vector.tensor_reduce(
                            ysb[:, j0 : j0 + w],
                            pr,
                            mybir.AxisListType.X,
                            OP.add,
                        )

            nc.sync.dma_start(y_pm[:], ysb[:])
            DEBUG_TILES.update(LG=LG, LH=LH, WG=WG, WH=WH, M4=M4,
                               ysb=ysb, m4d=m4d)

    nc.finalize()
    return nc


def _prep_inputs(x, core0, core1, core2, core3):
    """Host-side input marshalling: shard x over cores, lay out tensors in
    the on-chip layouts the kernel expects, pad core matrices for the
    shifted-corner matmuls (cast to bf16 on host)."""
    xs = np.ascontiguousarray(np.asarray(x, dtype=np.float32).reshape(NCORES, BS, 4))

    core0 = np.asarray(core0, dtype=np.float32)
    core1 = np.asarray(core1, dtype=np.float32)
    core2 = np.asarray(core2, dtype=np.float32)
    core3 = np.asarray(core3, dtype=np.float32)

    c0 = core0[0]                        # [128, 16]
    c0t = np.ascontiguousarray(
        np.concatenate([c0.T, c0.T[:, -1:]], axis=1)
    ).astype(ml_dtypes.bfloat16)
    c1 = core1.reshape(16, 2048)
    c1f = np.ascontiguousarray(
        np.concatenate([c1, np.tile(c1[:, -16:], (1, 3))], axis=1)
    ).astype(ml_dtypes.bfloat16)
    c2 = np.ascontiguousarray(core2.transpose(2, 1, 0)).reshape(16, 2048)
    c2t = np.ascontiguousarray(
        np.concatenate([c2, np.tile(c2[:, -16:], (1, 3))], axis=1)
    ).astype(ml_dtypes.bfloat16)
    c3 = core3[:, :, 0]                  # [16, 128]
    c3f = np.ascontiguousarray(
        np.concatenate([c3, c3[:, -1:]], axis=1)
    ).astype(ml_dtypes.bfloat16)

    in_maps = []
    for c in range(NCORES):
        xc_ = xs[c]
        x_pm = np.ascontiguousarray(
            xc_.reshape(J, P, 4).transpose(1, 0, 2)
        )  # [128, 256, 4]
        xg = np.ascontiguousarray(
            xc_[:, [0, 1]].reshape(M16, 16, 2).transpose(1, 0, 2)
        )  # [16, 2048, 2]
        xh = np.ascontiguousarray(
            xc_[:, [3, 2]].reshape(M16, 16, 2).transpose(1, 0, 2)
        )
        H16 = M16 // 2
        xq = np.concatenate(
            [xg[:, :H16], xg[:, H16:], xh[:, :H16], xh[:, H16:]], axis=0
        )  # [64, 1024, 2]
        in_maps.append(
            {
                "x_pm": x_pm,
                "xq": xq,
                "c0t": c0t,
                "c1f": c1f,
                "c3f": c3f,
                "c2t": c2t,
            }
        )
    return in_maps


def kernel(x, core0, core1, core2, core3):
    global _CACHED
    if _CACHED is None:
        _CACHED = _build_nc()
    nc = _CACHED
    in_maps = _prep_inputs(x, core0, core1, core2, core3)
    res = run_bass_kernel_spmd(nc, in_maps, core_ids=list(range(NCORES)))
    outs = []
    for c in range(NCORES):
        y_pm = res.results[c]["y_pm"]          # [128, 256]
        outs.append(np.ascontiguousarray(np.asarray(y_pm).T).reshape(-1))
    return np.concatenate(outs).astype(np.float32)


# revision 36
# speedup vs baseline: 1.0815x; 1.0676x over previous
"""Trainium2 Bass kernel for nn_ModelConTT_46016279609475 (TT interpolation).

y[b] = v0[b]^T V1[b] V2[b] v3[b], where v_i are linearly-interpolated slices
of tiny TT cores at per-point grid coordinates derived from x[b, :].

Strategy (per NeuronCore, data-parallel over B):
  * Precompute joint tables on device with PE matmuls (bf16 operands):
      G[n0, n1, k] = sum_c core0[n0, c] * core1[c, n1, k]      (u-side)
      H[n3, n2, k] = sum_c core3[c, n3] * core2[k, n2, c]      (v-side)
    stored fp16 in DRAM as 4-corner-packed 256B entries; the entry layout is
      G4[(n0*128+n1), (dn0, dn1, k)] fp16 in the first 128B, 128B pad,
    so one dma_gather element (256B minimum) fetches everything needed for
    the bilinear interpolation of u[b] (and same for v[b]).  fp16 halves the
    table-write DMA vs f32 and enables 2x-mode DVE in the combine.
  * Per point: idx = lo0*128 + lo1 (int16), one 256B dma_gather per table,
    then the 4-corner weighted sum and the final k-dot:
      y[b] = sum_k (sum_c wG_c gG[c,k]) * (sum_c wH_c gH[c,k])
    The corner weights are expanded over k into fp16 on the Activation
    engine (keeps every DVE op in 2x mode); the H-side corner multiply runs
    on GpSimd to keep the DVE stage under the gather-pair rate.

Batch mapping per core: shard b of size 32768; on-chip layout is
"p-minor": element i lives at partition i%128, free col i//128, matching
dma_gather's output layout dst[i%128, i//128]. Index lists are mod-16
wrapped as dma_gather requires (idx for i at [i%16, i//16]) and replicated
across all 8 Q7 core groups (each SWDGE core pair reads its own 16 rows).

Exact-floor trick (f32-safe): t = (xc + 2^23) - 2^23 rounds to nearest;
g = (t > xc); floor = t - g; frac = xc - floor computed via the exact
(t1 - 2^23) path to avoid re-rounding.
"""

import numpy as np
import ml_dtypes

import concourse.bass as bass
import concourse.bacc as bacc
import concourse.mybir as mybir
import concourse.tile as tile
from concourse import library_config
from concourse.bass_utils import run_bass_kernel_spmd

F32 = mybir.dt.float32
F16 = mybir.dt.float16
BF16 = mybir.dt.bfloat16
I16 = mybir.dt.int16
OP = mybir.AluOpType
AF = mybir.ActivationFunctionType

NCORES = 8
B = 262144
BS = B // NCORES          # 32768 points per core
P = 128                   # partitions
J = BS // P               # 256 free cols per partition
NCH = 8                   # pipeline chunks
JC = J // NCH             # 32 cols per chunk
NIDX = P * JC             # 4096 idxs per gather
LC = NIDX // 16           # 256 idx-list cols per chunk
N = 128                   # mode size
R = 16                    # TT rank
TE = N * N                # table entries
ES = 128                  # fp16 elems per entry: 4 corners x 16 k + 64 pad = 256B
ED = 64                   # fp16 elems of real data per entry
MAGIC = float(2 ** 23)
SCALE = (N - 1) / 2.0     # 63.5
M16 = BS // 16            # 2048 idx-list cols total

_CACHED = None
DEBUG_TILES = {}


def _build_nc(stage="full"):
    nc = bacc.Bacc("TRN2")

    x_pm = nc.dram_tensor("x_pm", [P, J, 4], F32, kind="ExternalInput")
    xq = nc.dram_tensor("xq", [64, M16 // 2, 2], F32, kind="ExternalInput")
    c0t = nc.dram_tensor("c0t", [16, 129], BF16, kind="ExternalInput")
    c1f = nc.dram_tensor("c1f", [16, 2096], BF16, kind="ExternalInput")
    c3f = nc.dram_tensor("c3f", [16, 129], BF16, kind="ExternalInput")
    c2t = nc.dram_tensor("c2t", [16, 2096], BF16, kind="ExternalInput")
    y_pm = nc.dram_tensor("y_pm", [P, J], F32, kind="ExternalOutput")

    with tile.TileContext(nc) as tc:
        with (
            tc.tile_pool(name="per", bufs=1) as pe,
            tc.tile_pool(name="ps", bufs=1, space="PSUM") as pp,
            tc.tile_pool(name="dr", bufs=1, space="DRAM") as dp,
        ):
            nc.gpsimd.load_library(library_config.mlp)

            # persistent tiles (lists fully memset once: the gather idx AP
            # spans all 128 partitions but HW only reads rows 0-31, its
            # queue's core pair; the sim reads rows 0-15)
            LG = pe.tile([P, M16], I16)
            LH = pe.tile([P, M16], I16)
            # no memsets: the gather idx APs span 128 partitions but HW only
            # reads rows 0-31 (its queue's core pair) and the sim rows 0-15,
            # all of which the band STTs + replication copies fully write.
            WG = pe.tile([P, 4, J], F32)
            WH = pe.tile([P, 4, J], F32)
            M4 = pe.tile([P, N, ES], F16)
            ysb = pe.tile([P, J], F32)
            # merged table: entry cell = [G-data 128B | H-data 128B]; the
            # H gather reads 256B starting at the H half (its tail spills
            # into the next entry's G half and is never read), so one extra
            # pad entry keeps the last H read in bounds.
            m4d = dp.tile([TE + 1, ES], F16)

            with tc.tile_pool(name="pre", bufs=1) as wp:
                # tiny warmup activation so LoadActFuncSet (1.3us) runs at
                # t=0 instead of right before the first table-pack copy.
                warm = wp.tile([1, 1], F32)
                nc.scalar.activation(warm[:], ysb[0:1, 0:1], AF.Copy,
                                     bias=0.0, scale=0.0)

                # ------------- constant loads -------------
                # core matrices first: they gate the PE matmuls -> Act packs
                # -> table writes, the longest head chain; the index lists
                # (from xq) have ~9us of slack behind it.
                c0t_s = wp.tile([16, 129], BF16)
                nc.sync.dma_start(c0t_s[:], c0t[:])
                c1f_s = wp.tile([16, 2096], BF16)
                nc.sync.dma_start(c1f_s[:], c1f[:])
                c3f_s = wp.tile([16, 129], BF16)
                nc.sync.dma_start(c3f_s[:], c3f[:])
                c2t_s = wp.tile([16, 2096], BF16)
                nc.sync.dma_start(c2t_s[:], c2t[:])
                xq_s = wp.tile([112, M16], F32)
                xqv = xq[:].rearrange("p a b -> p (a b)")
                nc.sync.dma_start(xq_s[0:16, :], xqv[0:16, :])
                nc.sync.dma_start(xq_s[32:48, :], xqv[16:32, :])
                nc.sync.dma_start(xq_s[64:80, :], xqv[32:48, :])
                nc.sync.dma_start(xq_s[96:112, :], xqv[48:64, :])

                # ------------- index lists (DVE + Act) ----
                # four 16-row bands (at partition bases 0/32/64/96 -- the
                # only legal compute starts): G cols 0-1023 / G cols
                # 1024-2047 / H cols 0-1023 / H cols 1024-2047. Halves the
                # per-op free size vs a single band.
                nc.vector.tensor_scalar(
                    xq_s[:], xq_s[:], SCALE, SCALE, OP.mult, OP.add
                )
                # t1q on DVE: Act is saturated by table packs.
                t1q = wp.tile([112, M16], F32)
                nc.vector.tensor_scalar(
                    t1q[:], xq_s[:], 1.0, MAGIC, OP.mult, OP.add
                )
                gq = wp.tile([112, M16], F32)
                nc.vector.scalar_tensor_tensor(
                    gq[:], t1q[:], -MAGIC, xq_s[:], OP.add, OP.is_gt
                )
                # lo = (t1 - MAGIC) - g  (exact floor), in place over t1q
                nc.vector.scalar_tensor_tensor(
                    t1q[:], t1q[:], -MAGIC, gq[:], OP.add, OP.subtract
                )
                # idx = lo_hi*128 + lo_lo, int16 cast fused into the op's
                # output dtype; written straight into the list tiles.
                lo_hi = t1q[:].rearrange("p (m two) -> p m two", two=2)
                H16 = M16 // 2
                for band, dst in (
                    (0, LG[0:16, 0:H16]),
                    (32, LG[32:48, H16:M16]),
                    (64, LH[64:80, 0:H16]),
                    (96, LH[96:112, H16:M16]),
                ):
                    nc.vector.scalar_tensor_tensor(
                        dst,
                        lo_hi[band : band + 16, :, 0],
                        128.0,
                        lo_hi[band : band + 16, :, 1],
                        OP.mult,
                        OP.add,
                    )
                # (the LG/LH replication copies are emitted after the table
                # build: they wait on the band STTs, and putting them ahead
                # of the table writes in the SP DMA queue head-of-line
                # blocks the writes for ~3us each)

                # ------------- table build ----------------
                # chunk-outer so each n1-quarter's DRAM write starts as soon
                # as its corner-pack copies land (overlaps write with build).
                # Entry cell (256B) = [G 4-corner c-major [c, k] fp16 128B |
                # H likewise 128B]: both tables in one dense DRAM write.
                # G packs on Activation, H packs on GpSimd (parallel);
                # DVE stays free for the index lists / interp weights.
                tgv = M4[:, :, 0:ED].rearrange("p n (c k) -> p n c k", k=R)
                thv = M4[:, :, ED:ES].rearrange("p n (c k) -> p n c k", k=R)
                tdrv = m4d[0:TE, :].rearrange("(p a) b -> p a b", p=P)
                for ch in range(4):
                    srcs = []
                    for tag, dst8, lhs, rhs in (
                        ("mmG", tgv, c0t_s, c1f_s),
                        ("mmH", thv, c3f_s, c2t_s),
                    ):
                        psq = pp.tile([P, 4, 512], F32, tag=tag)
                        for ci, (dhi, dlo) in enumerate(
                            ((0, 0), (0, 1), (1, 0), (1, 1))
                        ):
                            nc.tensor.matmul(
                                psq[:, ci],
                                lhs[:, dhi : dhi + 128],
                                rhs[
                                    :,
                                    16 * dlo + 512 * ch : 16 * dlo + 512 * ch + 512,
                                ],
                                start=True,
                                stop=True,
                            )
                        # pack: psq [p, c, (n1 32, k 16)] -> tbl [p, n1, c, k]
                        # (GPSIMD cannot read PSUM; all packs on Activation)
                        srcs.append(
                            (psq[:].rearrange("p c (a k) -> p a c k", k=R), dst8)
                        )
                    if ch < 3:
                        for src, dst8 in srcs:
                            nc.scalar.copy(dst8[:, 32 * ch : 32 * ch + 32], src)
                        nc.sync.dma_start(
                            tdrv[:, 32 * ch : 32 * ch + 32, :],
                            M4[:, 32 * ch : 32 * ch + 32, :],
                        )
                    else:
                        # last quarter in half-quarters (pack G, pack H,
                        # write): the final write gates the first gather's
                        # descriptor generation, so land it ~4us earlier.
                        for hh in range(2):
                            n0 = 32 * ch + 16 * hh
                            for src, dst8 in srcs:
                                nc.scalar.copy(
                                    dst8[:, n0 : n0 + 16],
                                    src[:, 16 * hh : 16 * hh + 16],
                                )
                            nc.sync.dma_start(
                                tdrv[:, n0 : n0 + 16, :],
                                M4[:, n0 : n0 + 16, :],
                            )

                # index-list replication on the gpsimd SWDGE queue: the SP
                # HWDGE queue carries the table writes, and these copies
                # (which gate the first gather's descriptor generation)
                # must not queue behind them.
                nc.gpsimd.dma_start(LG[0:16, H16:M16], LG[32:48, H16:M16])
                nc.gpsimd.dma_start(LG[16:32, :], LG[0:16, :])
                nc.gpsimd.dma_start(LH[0:16, 0:H16], LH[64:80, 0:H16])
                nc.gpsimd.dma_start(LH[0:16, H16:M16], LH[96:112, H16:M16])
                nc.gpsimd.dma_start(LH[16:32, :], LH[0:16, :])

                # ------------- gather + combine ---------------
                # Software-pipelined emission: the first gather pair is
                # emitted BEFORE the interp-weight chain so the Tile
                # framework's coarse semaphore batching doesn't make the
                # gathers wait on the (irrelevant) weight computation.
                if stage != "full":
                    nc.vector.memset(ysb[:], 0.0)
                # full-width chunks except the last, split 16/8/8 so the
                # post-final-gather combine tail is short.
                segs = [(JC * c, JC) for c in range(NCH - 1)]
                j4 = JC * (NCH - 1)
                segs += [(j4, 16), (j4 + 16, 8), (j4 + 24, 8)]
                if stage == "tables":
                    segs = []
                elif stage == "gather1":
                    segs = segs[:1]
                gsrc = m4d[0:TE, :]
                hsrc = (
                    m4d[:]
                    .rearrange("a b -> (a b)")[ED : ED + TE * ES]
                    .rearrange("(a b) -> a b", b=ES)
                )
                with (
                    tc.tile_pool(name="gbuf", bufs=3) as gb,
                    tc.tile_pool(name="cbuf", bufs=2) as cb,
                ):
                    gathered = []

                    def emit_gather(si):
                        j0, w = segs[si]
                        nidx = P * w
                        lc0, lc1 = 8 * j0, 8 * (j0 + w)
                        gGt = gb.tile([P, JC, ES], F16, tag="gG")
                        nc.gpsimd.dma_gather(
                            gGt[:, 0:w], gsrc, LG[:, lc0:lc1], nidx, nidx, ES,
                            queue_num=0, single_packet=False,
                        )
                        gHt = gb.tile([P, JC, ES], F16, tag="gH")
                        nc.gpsimd.dma_gather(
                            gHt[:, 0:w], hsrc, LH[:, lc0:lc1], nidx, nidx, ES,
                            queue_num=0, single_packet=False,
                        )
                        gathered.append((gGt, gHt))

                    if segs:
                        emit_gather(0)

                    # ------------- interp weights -------------
                    # x_s is [128, (256 j, 4 d)]; w = frac(xc), a = 1 - w.
                    # (loaded here, after the build, so its DMA doesn't
                    # delay the first table-quarter write)
                    x_s = wp.tile([P, J * 4], F32)
                    nc.sync.dma_start(x_s[:], x_pm[:].rearrange("p a b -> p (a b)"))
                    nc.vector.tensor_scalar(
                        x_s[:], x_s[:], SCALE, SCALE, OP.mult, OP.add
                    )
                    t1 = wp.tile([P, J * 4], F32)
                    nc.scalar.activation(
                        t1[:], x_s[:], AF.Copy, bias=MAGIC, scale=1.0
                    )
                    gw = wp.tile([P, J * 4], F32)
                    nc.vector.scalar_tensor_tensor(
                        gw[:], t1[:], -MAGIC, x_s[:], OP.add, OP.is_gt
                    )
                    # s1 = (t1 - MAGIC) - xc = t - xc  (t1 - MAGIC is exact)
                    s1 = wp.tile([P, J * 4], F32)
                    nc.vector.scalar_tensor_tensor(
                        s1[:], t1[:], -MAGIC, x_s[:], OP.add, OP.subtract
                    )
                    # w = g - (t - xc) = xc - floor(xc), in place over s1
                    nc.vector.tensor_tensor(s1[:], gw[:], s1[:], OP.subtract)
                    aw = wp.tile([P, J * 4], F32, tag="t1")
                    nc.vector.tensor_scalar(
                        aw[:], s1[:], -1.0, 1.0, OP.mult, OP.add
                    )

                    wv = s1[:].rearrange("p (j d) -> p j d", d=4)
                    av = aw[:].rearrange("p (j d) -> p j d", d=4)
                    # G corners (dn0, dn1): (a0,a1),(a0,w1),(w0,a1),(w0,w1)
                    nc.vector.tensor_tensor(WG[:, 0, :], av[:, :, 0], av[:, :, 1], OP.mult)
                    nc.vector.tensor_tensor(WG[:, 1, :], av[:, :, 0], wv[:, :, 1], OP.mult)
                    nc.vector.tensor_tensor(WG[:, 2, :], wv[:, :, 0], av[:, :, 1], OP.mult)
                    nc.vector.tensor_tensor(WG[:, 3, :], wv[:, :, 0], wv[:, :, 1], OP.mult)
                    # H corners (dn3, dn2): (a3,a2),(a3,w2),(w3,a2),(w3,w2)
                    nc.vector.tensor_tensor(WH[:, 0, :], av[:, :, 3], av[:, :, 2], OP.mult)
                    nc.vector.tensor_tensor(WH[:, 1, :], av[:, :, 3], wv[:, :, 2], OP.mult)
                    nc.vector.tensor_tensor(WH[:, 2, :], wv[:, :, 3], av[:, :, 2], OP.mult)
                    nc.vector.tensor_tensor(WH[:, 3, :], wv[:, :, 3], wv[:, :, 2], OP.mult)

                    for si, (j0, w) in enumerate(segs):
                        if si + 1 < len(segs):
                            emit_gather(si + 1)
                        gGt, gHt = gathered[si]

                        # fp16 weights expanded over k on Activation (keeps
                        # the DVE multiplies in 2x mode: packed stride-1).
                        uv = []
                        for ti, (g, W) in enumerate(((gGt, WG), (gHt, WH))):
                            wxt = cb.tile([P, JC, 4, R], F16, tag=f"wx{ti}")
                            wx = wxt[:, 0:w]
                            nc.scalar.copy(
                                wx,
                                W[:, :, j0 : j0 + w]
                                .rearrange("p c j -> p j c")
                                .unsqueeze(3)
                                .broadcast_to([P, w, 4, R]),
                            )
                            # m[j, c, k] = corner value * corner weight
                            gv = g[:, 0:w, 0:ED].rearrange(
                                "p j (c k) -> p j c k", c=4
                            )
                            mt = cb.tile([P, JC, 4, R], F16, tag=f"m{ti}")
                            m = mt[:, 0:w]
                            nc.vector.tensor_tensor(m, gv, wx, OP.mult)
                            t2t = cb.tile([P, JC, 2, R], F16, tag=f"t{ti}")
                            t2 = t2t[:, 0:w]
                            nc.vector.tensor_tensor(
                                t2, m[:, :, 0:2], m[:, :, 2:4], OP.add
                            )
                            ut = cb.tile([P, JC, R], F16, tag=f"u{ti}")
                            u = ut[:, 0:w]
                            nc.vector.tensor_tensor(
                                u, t2[:, :, 0], t2[:, :, 1], OP.add
                            )
                            uv.append(u)

                        prt = cb.tile([P, JC, R], F16, tag="pr")
                        pr = prt[:, 0:w]
                        nc.vector.tensor_tensor(pr, uv[0], uv[1], OP.mult)
                        nc.vector.tensor_reduce(
                            ysb[:, j0 : j0 + w],
                            pr,
                            mybir.AxisListType.X,
                            OP.add,
                        )

            nc.sync.dma_start(y_pm[:], ysb[:])
            DEBUG_TILES.update(LG=LG, LH=LH, WG=WG, WH=WH, M4=M4,
                               ysb=ysb, m4d=m4d)

    nc.finalize()
    return nc


def _prep_inputs(x, core0, core1, core2, core3):
    """Host-side input marshalling: shard x over cores, lay out tensors in
    the on-chip layouts the kernel expects, pad core matrices for the
    shifted-corner matmuls (cast to bf16 on host)."""
    xs = np.ascontiguousarray(np.asarray(x, dtype=np.float32).reshape(NCORES, BS, 4))

    core0 = np.asarray(core0, dtype=np.float32)
    core1 = np.asarray(core1, dtype=np.float32)
    core2 = np.asarray(core2, dtype=np.float32)
    core3 = np.asarray(core3, dtype=np.float32)

    c0 = core0[0]                        # [128, 16]
    c0t = np.ascontiguousarray(
        np.concatenate([c0.T, c0.T[:, -1:]], axis=1)
    ).astype(ml_dtypes.bfloat16)
    c1 = core1.reshape(16, 2048)
    c1f = np.ascontiguousarray(
        np.concatenate([c1, np.tile(c1[:, -16:], (1, 3))], axis=1)
    ).astype(ml_dtypes.bfloat16)
    c2 = np.ascontiguousarray(core2.transpose(2, 1, 0)).reshape(16, 2048)
    c2t = np.ascontiguousarray(
        np.concatenate([c2, np.tile(c2[:, -16:], (1, 3))], axis=1)
    ).astype(ml_dtypes.bfloat16)
    c3 = core3[:, :, 0]                  # [16, 128]
    c3f = np.ascontiguousarray(
        np.concatenate([c3, c3[:, -1:]], axis=1)
    ).astype(ml_dtypes.bfloat16)

    in_maps = []
    for c in range(NCORES):
        xc_ = xs[c]
        x_pm = np.ascontiguousarray(
            xc_.reshape(J, P, 4).transpose(1, 0, 2)
        )  # [128, 256, 4]
        xg = np.ascontiguousarray(
            xc_[:, [0, 1]].reshape(M16, 16, 2).transpose(1, 0, 2)
        )  # [16, 2048, 2]
        xh = np.ascontiguousarray(
            xc_[:, [3, 2]].reshape(M16, 16, 2).transpose(1, 0, 2)
        )
        H16 = M16 // 2
        xq = np.concatenate(
            [xg[:, :H16], xg[:, H16:], xh[:, :H16], xh[:, H16:]], axis=0
        )  # [64, 1024, 2]
        in_maps.append(
            {
                "x_pm": x_pm,
                "xq": xq,
                "c0t": c0t,
                "c1f": c1f,
                "c3f": c3f,
                "c2t": c2t,
            }
        )
    return in_maps


def kernel(x, core0, core1, core2, core3):
    global _CACHED
    if _CACHED is None:
        _CACHED = _build_nc()
    nc = _CACHED
    in_maps = _prep_inputs(x, core0, core1, core2, core3)
    res = run_bass_kernel_spmd(nc, in_maps, core_ids=list(range(NCORES)))
    outs = []
    for c in range(NCORES):
        y_pm = res.results[c]["y_pm"]          # [128, 256]
        outs.append(np.ascontiguousarray(np.asarray(y_pm).T).reshape(-1))
    return np.concatenate(outs).astype(np.float32)
